# revision 19
# baseline (speedup 1.0000x reference)
"""Trainium2 Bass kernel for DenoisedSignalFeatureExtractor.

Data-parallel over 8 NeuronCores: each core runs 4 of the 32 batch samples
through the full network (conv stem -> 2 mamba blocks -> channel attention ->
pooling). Weights are replicated; all x-dependent compute runs on device.

Layout convention: features on partitions, time on the free dimension.
The selective scan runs as hardware `tensor_tensor_scan` per (d-half, state n)
with exp(A*dt) built on the scalar engine via its per-partition scale operand.
"""
import sys
sys.path.insert(0, "/opt/trn_rl_repo")

import numpy as np
from contextlib import ExitStack

import concourse.bass as bass
import concourse.bacc as bacc
import concourse.mybir as mybir
import concourse.tile as tile
from concourse import bass_utils

F32 = mybir.dt.float32
BF16 = mybir.dt.bfloat16
AF = mybir.ActivationFunctionType
ALU = mybir.AluOpType
AX = mybir.AxisListType

# Model dims (fixed by the problem)
B_TOTAL, L = 32, 1024
D_IN, DM = 32, 128            # input channels, d_model
DI, DS, DC, DTR = 256, 16, 4, 8  # d_inner, d_state, d_conv, dt_rank
POS_LEN = 128
N_CORES = 8
B_LOC = B_TOTAL // N_CORES    # samples per core
TC = 512                      # t-chunk for PSUM-bound matmuls


# ---------------------------------------------------------------- constants

def _np(a):
    return np.asarray(a, dtype=np.float32)


def _interp_pos(pos, Lx):
    P = pos.shape[0]
    src = (np.arange(Lx, dtype=np.float32) + 0.5) * (P / Lx) - 0.5
    src = np.clip(src, 0.0, P - 1.0)
    i0 = np.floor(src).astype(np.int32)
    i1 = np.minimum(i0 + 1, P - 1)
    w = (src - i0)[:, None].astype(np.float32)
    return pos[i0] * (1.0 - w) + pos[i1] * w


def prep_consts(params):
    """All parameter-derived constants, shaped for the kernel's DRAM inputs."""
    c = {}
    pr = params['proj']
    w = _np(pr['w'])                       # (128, 32, 7)
    # stem taps as lhsT [32, 128] per tap, concatenated -> [32, 7*128]
    c['stem_w'] = np.concatenate([w[:, :, k].T for k in range(7)], axis=1).astype(np.float32)
    g, bb = _np(pr['bn_g']), _np(pr['bn_b'])
    c['stem_scale'] = g.reshape(DM, 1)
    c['stem_bias'] = (g * _np(pr['b']) + bb).reshape(DM, 1)
    c['pe'] = _interp_pos(_np(params['pos'][0]), L).T.copy()       # (128, L)

    for l, bp in enumerate(params['blocks']):
        W_in = _np(bp['in_proj'])          # (512, 128)
        c[f'win{l}'] = W_in.T.copy()       # (128, 512) lhsT
        cw = _np(bp['conv_w'])[:, 0, :]                            # (256, 4)
        c[f'convw{l}'] = np.concatenate([cw[:DM], cw[DM:]], axis=1)  # (128, 8)
        cb = _np(bp['conv_b']).reshape(DI, 1)
        c[f'convb{l}'] = np.concatenate([cb[:DM], cb[DM:]], axis=1)  # (128, 2)
        Wx = _np(bp['x_proj'])             # (40, 256)
        wxT = Wx.T                         # (256, 40)
        # pad output rows to partition-aligned bases: dt_lin@0, B@32, C@64
        wxp = np.zeros((DI, 96), np.float32)
        wxp[:, 0:DTR] = wxT[:, 0:DTR]
        wxp[:, 32:32 + DS] = wxT[:, DTR:DTR + DS]
        wxp[:, 64:64 + DS] = wxT[:, DTR + DS:DTR + 2 * DS]
        c[f'wx{l}'] = np.concatenate([wxp[:DM], wxp[DM:]], axis=1)   # (128, 192)
        Wdt = _np(bp['dt_w'])              # (256, 8)
        c[f'wdt{l}'] = Wdt.T.copy()        # (8, 256) lhsT
        db = _np(bp['dt_b']).reshape(DI, 1)
        c[f'dtb{l}'] = np.concatenate([db[:DM], db[DM:]], axis=1)    # (128, 2)
        Am = -np.exp(_np(bp['A_log']))                             # (256, 16)
        c[f'A{l}'] = np.concatenate([Am[:DM], Am[DM:]], axis=1)      # (128, 32)
        dv = _np(bp['D']).reshape(DI, 1)
        c[f'Dv{l}'] = np.concatenate([dv[:DM], dv[DM:]], axis=1)     # (128, 2)
        Wo = _np(bp['out_proj'])           # (128, 256)
        woT = Wo.T                         # (256, 128)
        c[f'wout{l}'] = np.concatenate([woT[:DM], woT[DM:]], axis=1) # (128, 256)
        c[f'ln1g{l}'] = _np(bp['ln1_g']).reshape(DM, 1)
        c[f'ln1b{l}'] = _np(bp['ln1_b']).reshape(DM, 1)
        c[f'ln2g{l}'] = _np(bp['ln2_g']).reshape(DM, 1)
        c[f'ln2b{l}'] = _np(bp['ln2_b']).reshape(DM, 1)
        tw = _np(bp['te_w'])               # (128, 32, 3) groups=4
        WBD = np.zeros((DM, DM, 3), np.float32)
        for gi in range(4):
            WBD[gi*32:(gi+1)*32, gi*32:(gi+1)*32, :] = tw[gi*32:(gi+1)*32]
        c[f'tew{l}'] = np.concatenate([WBD[:, :, k].T for k in range(3)], axis=1)
        tg = _np(bp['te_bn_g'])
        c[f'tes{l}'] = tg.reshape(DM, 1)
        c[f'teb{l}'] = (tg * _np(bp['te_b']) + _np(bp['te_bn_b'])).reshape(DM, 1)

    ca = params['ca']
    c['caw1'] = _np(ca['w1']).T.copy()     # (128, 16) lhsT
    c['caw2'] = _np(ca['w2']).T.copy()     # (16, 128) lhsT
    ew = _np(ca['enh_w'])                  # (128, 4, 3) groups=32
    EBD = np.zeros((DM, DM, 3), np.float32)
    for gi in range(32):
        EBD[gi*4:(gi+1)*4, gi*4:(gi+1)*4, :] = ew[gi*4:(gi+1)*4]
    c['caew'] = np.concatenate([EBD[:, :, k].T for k in range(3)], axis=1)
    cg = _np(ca['bn_g'])
    c['cas'] = cg.reshape(DM, 1)
    c['cab'] = (cg * _np(ca['enh_b']) + _np(ca['bn_b'])).reshape(DM, 1)

    ae = params['ae']
    aw = _np(ae['w'])                      # (128, 128, 3)
    c['aew'] = np.concatenate([aw[:, :, k].T for k in range(3)], axis=1)
    ag = _np(ae['bn_g'])
    c['aes'] = ag.reshape(DM, 1)
    c['aeb'] = (ag * _np(ae['b']) + _np(ae['bn_b'])).reshape(DM, 1)

    # selector matrices: sel_n = [16, 128] with row n all ones, concatenated
    sel = np.zeros((DS, DS * DM), np.float32)
    for n in range(DS):
        sel[n, n*DM:(n+1)*DM] = 1.0
    c['sel'] = sel
    c['ones_col'] = np.ones((DM, 1), np.float32)    # lhsT for partition-sum
    c['ones_row'] = np.ones((1, DM), np.float32)    # lhsT for row-broadcast
    import ml_dtypes
    for k in BF_CONSTS:
        c[k] = c[k].astype(ml_dtypes.bfloat16)
    return c


CONST_SHAPES = None  # filled lazily
BF_CONSTS = {'sel', 'win0', 'win1', 'wx0', 'wx1', 'wdt0', 'wdt1', 'wout0', 'wout1'}


# ---------------------------------------------------------------- kernel body

def build_kernel(b_loc=B_LOC, num_scan_states=DS, nblocks=2, tail=3):
    """Builds the Bacc graph. Returns (nc, const_names)."""
    nc = bacc.Bacc("TRN2", debug=False, enable_asserts=False,
                   detect_race_conditions=False)
    consts = prep_consts_shapes()
    dts = {}
    for name, shape in consts.items():
        dt_ = BF16 if name in BF_CONSTS else F32
        dts[name] = nc.dram_tensor(name, list(shape), dt_, kind="ExternalInput").ap()
    x_in = nc.dram_tensor("x", [b_loc, D_IN, L], F32, kind="ExternalInput").ap()
    o_pooled = nc.dram_tensor("pooled", [b_loc, 3 * DM], F32, kind="ExternalOutput").ap()
    o_gf = nc.dram_tensor("gf", [b_loc, DM], F32, kind="ExternalOutput").ap()
    o_enh = nc.dram_tensor("enh", [b_loc, DM, L], F32, kind="ExternalOutput").ap()

    with tile.TileContext(nc) as tc:
        _body(tc, dts, x_in, o_pooled, o_gf, o_enh, b_loc, num_scan_states, nblocks, tail)
    nc.compile()
    return nc


def prep_consts_shapes():
    global CONST_SHAPES
    if CONST_SHAPES is None:
        CONST_SHAPES = {
            'stem_w': (D_IN, 7 * DM), 'stem_scale': (DM, 1), 'stem_bias': (DM, 1),
            'pe': (DM, L),
            'caw1': (DM, 16), 'caw2': (16, DM), 'caew': (DM, 3 * DM),
            'cas': (DM, 1), 'cab': (DM, 1),
            'aew': (DM, 3 * DM), 'aes': (DM, 1), 'aeb': (DM, 1),
            'sel': (DS, DS * DM), 'ones_col': (DM, 1), 'ones_row': (1, DM),
        }
        for l in range(2):
            CONST_SHAPES.update({
                f'win{l}': (DM, 2 * DI), f'convw{l}': (DM, 2 * DC), f'convb{l}': (DM, 2),
                f'wx{l}': (DM, 2 * 96), f'wdt{l}': (DTR, DI), f'dtb{l}': (DM, 2),
                f'A{l}': (DM, 2 * DS), f'Dv{l}': (DM, 2), f'wout{l}': (DM, 2 * DM),
                f'ln1g{l}': (DM, 1), f'ln1b{l}': (DM, 1),
                f'ln2g{l}': (DM, 1), f'ln2b{l}': (DM, 1),
                f'tew{l}': (DM, 3 * DM), f'tes{l}': (DM, 1), f'teb{l}': (DM, 1),
            })
    return CONST_SHAPES


def _body(tc, dts, x_in, o_pooled, o_gf, o_enh, b_loc, NS, nblocks=2, tail=3):
    nc = tc.nc
    ctx = ExitStack()
    with ctx:
        cpool = ctx.enter_context(tc.tile_pool(name="consts", bufs=1))
        sbp = ctx.enter_context(tc.tile_pool(name="work", bufs=1))
        sb1 = ctx.enter_context(tc.tile_pool(name="big", bufs=1))
        psp = ctx.enter_context(tc.tile_pool(name="ps", bufs=2, space="PSUM"))
        pss = ctx.enter_context(tc.tile_pool(name="ps_small", bufs=2, space="PSUM"))

        # ---- load constants to SBUF once
        ct = {}
        for name, shape in prep_consts_shapes().items():
            dt_ = BF16 if name in BF_CONSTS else F32
            t = cpool.tile(list(shape), dt_, tag=f"c_{name}", name=f"c_{name}")
            nc.sync.dma_start(t[:], dts[name][:])
            ct[name] = t

        NCH = L // TC  # chunks

        eps5 = cpool.tile([1, 1], F32, tag="eps5", name="eps5")
        nc.vector.memset(eps5[:], 1e-5)
        eps8 = cpool.tile([DM, 1], F32, tag="eps8", name="eps8")
        nc.vector.memset(eps8[:], 1e-8)

        def ln(src_tile, g_col, b_col, out_tile):
            """LayerNorm over the partition dim (128) of [128, L]."""
            mrow = sbp.tile([1, L], F32, tag="ln_mrow")
            sq = sbp.tile([DM, L], F32, tag="ln_sq")
            ctr = sbp.tile([DM, L], F32, tag="ln_ctr")
            rrow = sbp.tile([1, L], F32, tag="ln_rrow")
            for c in range(NCH):
                s = slice(c * TC, (c + 1) * TC)
                pm = pss.tile([1, TC], F32, tag="sm_ps")
                nc.tensor.matmul(pm[:], ct['ones_col'][:], src_tile[:, s],
                                 start=True, stop=True)
                nc.scalar.activation(mrow[:, s], pm[:], AF.Copy, scale=1.0 / DM)
                pb = psp.tile([DM, TC], F32, tag="mm_ps")
                nc.tensor.matmul(pb[:], ct['ones_row'][:], mrow[:, s],
                                 start=True, stop=True)
                nc.vector.tensor_sub(ctr[:, s], src_tile[:, s], pb[:])
                nc.scalar.activation(sq[:, s], ctr[:, s], AF.Square)
                pv = pss.tile([1, TC], F32, tag="sm_ps")
                nc.tensor.matmul(pv[:], ct['ones_col'][:], sq[:, s],
                                 start=True, stop=True)
                nc.scalar.activation(rrow[:, s], pv[:], AF.Ln,
                                     scale=1.0 / DM, bias=eps5[:])
                nc.scalar.activation(rrow[:, s], rrow[:, s], AF.Exp, scale=-0.5)
                pr = psp.tile([DM, TC], F32, tag="mm_ps")
                nc.tensor.matmul(pr[:], ct['ones_row'][:], rrow[:, s],
                                 start=True, stop=True)
                nc.vector.tensor_mul(sq[:, s], ctr[:, s], pr[:])
                nc.scalar.activation(out_tile[:, s], sq[:, s], AF.Identity,
                                     scale=g_col[:], bias=b_col[:])

        for b in range(b_loc):
            # ======== stem: conv7 + bn + gelu, + positional add
            xpad = sbp.tile([D_IN, L + 6], F32, tag="xpad")
            nc.vector.memset(xpad[:], 0.0)
            nc.sync.dma_start(xpad[:, 3:3 + L], x_in[b])
            tf = sbp.tile([DM, L], F32, tag="tf")
            for c in range(NCH):
                ps = psp.tile([DM, TC], F32, tag="mm_ps")
                for k in range(7):
                    nc.tensor.matmul(ps[:], ct['stem_w'][:, k*DM:(k+1)*DM],
                                     xpad[:, c*TC + k: c*TC + k + TC],
                                     start=(k == 0), stop=(k == 6))
                nc.scalar.activation(tf[:, c*TC:(c+1)*TC], ps[:], AF.Gelu,
                                     scale=ct['stem_scale'][:], bias=ct['stem_bias'][:])
            nc.vector.tensor_add(tf[:], tf[:], ct['pe'][:])

            # ======== mamba blocks
            for l in range(nblocks):
                tf_bf = sbp.tile([DM, L], BF16, tag="tf_bf")
                nc.scalar.activation(tf_bf[:], tf[:], AF.Copy)
                xr = [sbp.tile([DM, 3 + L], BF16, tag=f"xr{dh}", name=f"xr{dh}") for dh in range(2)]
                zs = [sbp.tile([DM, L], BF16, tag=f"zs{dh}", name=f"zs{dh}") for dh in range(2)]
                for dh in range(2):
                    nc.vector.memset(xr[dh][:, 0:3], 0.0)
                for c in range(NCH):
                    s = slice(c * TC, (c + 1) * TC)
                    for m in range(4):
                        ps = psp.tile([DM, TC], F32, tag="mm_ps")
                        nc.tensor.matmul(ps[:], ct[f'win{l}'][:, m*DM:(m+1)*DM],
                                         tf_bf[:, s], start=True, stop=True)
                        if m < 2:
                            nc.scalar.activation(xr[m][:, 3 + c*TC: 3 + (c+1)*TC],
                                                 ps[:], AF.Copy)
                        else:
                            nc.scalar.activation(zs[m - 2][:, s], ps[:], AF.Silu)

                # causal depthwise conv (k=4) + silu -> u
                u = [sbp.tile([DM, L], BF16, tag=f"u{dh}", name=f"u{dh}") for dh in range(2)]
                dt = [sbp.tile([DM, L], F32, tag=f"dt{dh}", name=f"dt{dh}") for dh in range(2)]
                dtu = [sbp.tile([DM, L], BF16, tag=f"dtu{dh}", name=f"dtu{dh}") for dh in range(2)]
                for dh in range(2):
                    acc0 = sbp.tile([DM, L], BF16, tag="cacc0")
                    acc1 = sbp.tile([DM, L], BF16, tag="cacc1")
                    cw = ct[f'convw{l}']
                    nc.vector.tensor_scalar(acc0[:], xr[dh][:, 0:L],
                                            cw[:, dh*DC:dh*DC+1], None,
                                            op0=ALU.mult)
                    nc.vector.scalar_tensor_tensor(acc1[:], xr[dh][:, 1:1+L],
                                                   cw[:, dh*DC+1:dh*DC+2], acc0[:],
                                                   op0=ALU.mult, op1=ALU.add)
                    nc.vector.scalar_tensor_tensor(acc0[:], xr[dh][:, 2:2+L],
                                                   cw[:, dh*DC+2:dh*DC+3], acc1[:],
                                                   op0=ALU.mult, op1=ALU.add)
                    nc.vector.scalar_tensor_tensor(acc1[:], xr[dh][:, 3:3+L],
                                                   cw[:, dh*DC+3:dh*DC+4], acc0[:],
                                                   op0=ALU.mult, op1=ALU.add)
                    nc.scalar.activation(u[dh][:], acc1[:], AF.Silu,
                                         bias=ct[f'convb{l}'][:, dh:dh+1])

                # x_proj -> dt_lin [8, L], B [16, L], C [16, L]
                dtl = sbp.tile([DTR, L], BF16, tag="dtl")
                Bm = sbp.tile([DS, L], BF16, tag="Bm")
                Cm = sbp.tile([DS, L], BF16, tag="Cm")
                for c in range(NCH):
                    s = slice(c * TC, (c + 1) * TC)
                    ps = pss.tile([96, TC], F32, tag="sm_ps")
                    for dh in range(2):
                        nc.tensor.matmul(ps[:], ct[f'wx{l}'][:, dh*96:(dh+1)*96],
                                         u[dh][:, s], start=(dh == 0), stop=(dh == 1))
                    nc.scalar.activation(dtl[:, s], ps[0:DTR, :], AF.Copy)
                    nc.scalar.activation(Bm[:, s], ps[32:32+DS, :], AF.Copy)
                    nc.scalar.activation(Cm[:, s], ps[64:64+DS, :], AF.Copy)

                # dt = softplus(Wdt @ dt_lin + dt_b) via exp/ln (one table set)
                esbs = []
                for dh in range(2):
                    esb = sbp.tile([DM, L], F32, tag=f"dt_esb{dh}", name=f"esb{dh}")
                    for c in range(NCH):
                        s = slice(c * TC, (c + 1) * TC)
                        ps = psp.tile([DM, TC], F32, tag="mm_ps")
                        nc.tensor.matmul(ps[:], ct[f'wdt{l}'][:, dh*DM:(dh+1)*DM],
                                         dtl[:, s], start=True, stop=True)
                        nc.scalar.activation(esb[:, s], ps[:], AF.Exp,
                                             bias=ct[f'dtb{l}'][:, dh:dh+1])
                    nc.vector.tensor_scalar_add(esb[:], esb[:], 1.0)
                    esbs.append(esb)
                for dh in range(2):
                    nc.scalar.activation(dt[dh][:], esbs[dh][:], AF.Ln)
                    nc.vector.tensor_mul(dtu[dh][:], dt[dh][:], u[dh][:])

                # ---- selective scan: broadcasts shared across dh, bf16 h
                y = [sbp.tile([DM, L], F32, tag=f"y{dh}", name=f"my{dh}") for dh in range(2)]
                hbufs = [sb1.tile([DM, L * NS], BF16, tag=f"hbuf{dh}", name=f"hbuf{dh}")
                         for dh in range(2)]
                hviews = [h.rearrange("p (t n) -> p t n", n=NS) for h in hbufs]
                for n in range(NS):
                    pb = psp.tile([DM, L], F32, tag="bc_ps", bufs=2)
                    for c in range(NCH):
                        s = slice(c * TC, (c + 1) * TC)
                        nc.tensor.matmul(pb[:, s], ct['sel'][:, n*DM:(n+1)*DM],
                                         Bm[:, s], start=True, stop=True)
                    for dh in range(2):
                        dA = sbp.tile([DM, L], F32, tag="dA", bufs=3)
                        nc.scalar.activation(
                            dA[:], dt[dh][:], AF.Exp,
                            scale=ct[f'A{l}'][:, dh*DS + n: dh*DS + n + 1])
                        w = sbp.tile([DM, L], BF16, tag="w", bufs=3)
                        nc.vector.tensor_mul(w[:], dtu[dh][:], pb[:])
                        nc.vector.tensor_tensor_scan(
                            hviews[dh][:, :, n], dA[:], w[:],
                            0.0, op0=ALU.mult, op1=ALU.add)
                for n in range(NS):
                    pc = psp.tile([DM, L], F32, tag="bc_ps", bufs=2)
                    for c in range(NCH):
                        s = slice(c * TC, (c + 1) * TC)
                        nc.tensor.matmul(pc[:, s], ct['sel'][:, n*DM:(n+1)*DM],
                                         Cm[:, s], start=True, stop=True)
                    for dh in range(2):
                        nc.vector.tensor_mul(hviews[dh][:, :, n], hviews[dh][:, :, n], pc[:])
                for dh in range(2):
                    nc.vector.tensor_reduce(y[dh][:], hviews[dh][:], axis=AX.X, op=ALU.add)

                # y = (y + u*D) * silu(z); out_proj; ln1; residual; ln2
                mo = sbp.tile([DM, L], F32, tag="mo")
                yb = [sbp.tile([DM, L], BF16, tag=f"yb{dh}", name=f"yb{dh}") for dh in range(2)]
                for dh in range(2):
                    nc.vector.scalar_tensor_tensor(
                        y[dh][:], u[dh][:], ct[f'Dv{l}'][:, dh:dh+1], y[dh][:],
                        op0=ALU.mult, op1=ALU.add)
                    nc.vector.tensor_mul(yb[dh][:], y[dh][:], zs[dh][:])
                for c in range(NCH):
                    s = slice(c * TC, (c + 1) * TC)
                    ps = psp.tile([DM, TC], F32, tag="mm_ps")
                    for dh in range(2):
                        nc.tensor.matmul(ps[:], ct[f'wout{l}'][:, dh*DM:(dh+1)*DM],
                                         yb[dh][:, s], start=(dh == 0), stop=(dh == 1))
                    nc.scalar.activation(mo[:, s], ps[:], AF.Copy)
                y1 = sbp.tile([DM, L], F32, tag="y1")
                ln(mo, ct[f'ln1g{l}'], ct[f'ln1b{l}'], y1)
                nc.vector.tensor_add(y1[:], y1[:], tf[:])
                x2p = sbp.tile([DM, L + 2], F32, tag="x2p")
                nc.vector.memset(x2p[:, 0:1], 0.0)
                nc.vector.memset(x2p[:, L+1:L+2], 0.0)
                x2 = x2p[:, 1:1+L]
                ln(y1, ct[f'ln2g{l}'], ct[f'ln2b{l}'], x2)
                # te conv (block-diag, k=3, pad 1) + gelu + residual
                tfn = sbp.tile([DM, L], F32, tag="ln_sq")
                for c in range(NCH):
                    ps = psp.tile([DM, TC], F32, tag="mm_ps")
                    for k in range(3):
                        nc.tensor.matmul(ps[:], ct[f'tew{l}'][:, k*DM:(k+1)*DM],
                                         x2p[:, c*TC + k: c*TC + k + TC],
                                         start=(k == 0), stop=(k == 2))
                    nc.scalar.activation(tfn[:, c*TC:(c+1)*TC], ps[:], AF.Gelu,
                                         scale=ct[f'tes{l}'][:], bias=ct[f'teb{l}'][:])
                nc.vector.tensor_add(tf[:], tfn[:], x2)

            if tail == 0:
                nc.sync.dma_start(o_enh[b], tf[:])
                zz = sbp.tile([DM, 3], F32, tag="zz")
                nc.vector.memset(zz[:], 0.0)
                nc.sync.dma_start(o_pooled[b, 0:DM], zz[:, 0])
                nc.sync.dma_start(o_pooled[b, DM:2*DM], zz[:, 1])
                nc.sync.dma_start(o_pooled[b, 2*DM:3*DM], zz[:, 2])
                nc.sync.dma_start(o_gf[b], zz[:, 0])
                continue
            # ======== channel attention
            avg = sbp.tile([DM, 1], F32, tag="avg")
            mx = sbp.tile([DM, 1], F32, tag="mx")
            nc.vector.tensor_reduce(avg[:], tf[:], axis=AX.X, op=ALU.add)
            nc.scalar.activation(avg[:], avg[:], AF.Copy, scale=1.0 / L)
            nc.vector.tensor_reduce(mx[:], tf[:], axis=AX.X, op=ALU.max)
            att_ps = pss.tile([DM, 1], F32, tag="sm_ps")
            for i, v in enumerate((avg, mx)):
                ph = pss.tile([16, 1], F32, tag="sm_ps")
                nc.tensor.matmul(ph[:], ct['caw1'][:], v[:], start=True, stop=True)
                hg = sbp.tile([16, 1], F32, tag="cahg")
                nc.scalar.activation(hg[:], ph[:], AF.Gelu)
                nc.tensor.matmul(att_ps[:], ct['caw2'][:], hg[:],
                                 start=(i == 0), stop=(i == 1))
            a_sig = sbp.tile([DM, 1], F32, tag="asig")
            nc.scalar.activation(a_sig[:], att_ps[:], AF.Exp, scale=-1.0)
            nc.vector.tensor_scalar_add(a_sig[:], a_sig[:], 1.0)
            nc.vector.reciprocal(a_sig[:], a_sig[:])
            xap = sbp.tile([DM, L + 2], F32, tag="xr0")
            nc.vector.memset(xap[:, 0:1], 0.0)
            nc.vector.memset(xap[:, L+1:L+2], 0.0)
            nc.vector.tensor_scalar(xap[:, 1:1+L], tf[:], a_sig[:], None, op0=ALU.mult)
            attp = sbp.tile([DM, L + 2], F32, tag="xr1")
            nc.vector.memset(attp[:, 0:1], 0.0)
            nc.vector.memset(attp[:, L+1:L+2], 0.0)
            for c in range(NCH):
                ps = psp.tile([DM, TC], F32, tag="mm_ps")
                for k in range(3):
                    nc.tensor.matmul(ps[:], ct['caew'][:, k*DM:(k+1)*DM],
                                     xap[:, c*TC + k: c*TC + k + TC],
                                     start=(k == 0), stop=(k == 2))
                xe = sbp.tile([DM, TC], F32, tag="dtu0")
                nc.scalar.activation(xe[:], ps[:], AF.Gelu,
                                     scale=ct['cas'][:], bias=ct['cab'][:])
                nc.vector.scalar_tensor_tensor(attp[:, 1 + c*TC: 1 + (c+1)*TC],
                                               xe[:], 0.1,
                                               xap[:, 1 + c*TC: 1 + (c+1)*TC],
                                               op0=ALU.mult, op1=ALU.add)
            # ae conv + gelu + residual -> enh
            enh_t = sbp.tile([DM, L], F32, tag="u0")
            for c in range(NCH):
                ps = psp.tile([DM, TC], F32, tag="mm_ps")
                for k in range(3):
                    nc.tensor.matmul(ps[:], ct['aew'][:, k*DM:(k+1)*DM],
                                     attp[:, c*TC + k: c*TC + k + TC],
                                     start=(k == 0), stop=(k == 2))
                nc.scalar.activation(enh_t[:, c*TC:(c+1)*TC], ps[:], AF.Gelu,
                                     scale=ct['aes'][:], bias=ct['aeb'][:])
            nc.vector.tensor_add(enh_t[:], enh_t[:], attp[:, 1:1+L])
            nc.sync.dma_start(o_enh[b], enh_t[:])
            if tail <= 2:
                zz = sbp.tile([DM, 3], F32, tag="zz")
                nc.vector.memset(zz[:], 0.0)
                nc.sync.dma_start(o_pooled[b, 0:DM], zz[:, 0])
                nc.sync.dma_start(o_pooled[b, DM:2*DM], zz[:, 1])
                nc.sync.dma_start(o_pooled[b, 2*DM:3*DM], zz[:, 2])
                nc.sync.dma_start(o_gf[b], zz[:, 0])
                continue

            # ======== pooling: gf (mean), mx, std
            gf_s = sbp.tile([DM, 1], F32, tag="gfs")
            nc.vector.tensor_reduce(gf_s[:], enh_t[:], axis=AX.X, op=ALU.add)
            gf_m = sbp.tile([DM, 1], F32, tag="gfm")
            nc.scalar.activation(gf_m[:], gf_s[:], AF.Copy, scale=1.0 / L)
            nc.sync.dma_start(o_gf[b], gf_m[:, 0])
            nc.sync.dma_start(o_pooled[b, 0:DM], gf_m[:, 0])
            mx2 = sbp.tile([DM, 1], F32, tag="mx2")
            nc.vector.tensor_reduce(mx2[:], enh_t[:], axis=AX.X, op=ALU.max)
            nc.sync.dma_start(o_pooled[b, DM:2*DM], mx2[:, 0])
            sq_scr = sbp.tile([DM, L], F32, tag="u1")
            nc.scalar.activation(sq_scr[:], enh_t[:], AF.Square)
            ssq = sbp.tile([DM, 1], F32, tag="ssq")
            nc.vector.tensor_reduce(ssq[:], sq_scr[:], axis=AX.X, op=ALU.add)
            gq = sbp.tile([DM, 1], F32, tag="gq")
            nc.scalar.activation(gq[:], gf_m[:], AF.Square)
            var = sbp.tile([DM, 1], F32, tag="var")
            nc.vector.scalar_tensor_tensor(var[:], ssq[:], 1.0 / L, gq[:],
                                           op0=ALU.mult, op1=ALU.subtract)
            nc.vector.tensor_scalar_max(var[:], var[:], 0.0)
            std = sbp.tile([DM, 1], F32, tag="std")
            nc.scalar.activation(std[:], var[:], AF.Ln, bias=eps8[:])
            nc.scalar.activation(std[:], std[:], AF.Exp, scale=0.5)
            nc.sync.dma_start(o_pooled[b, 2*DM:3*DM], std[:, 0])


# ---------------------------------------------------------------- entrypoint

_CACHE = {}


def kernel(x, params):
    x = np.asarray(x, dtype=np.float32)
    consts = prep_consts(params)
    if "nc" not in _CACHE:
        _CACHE["nc"] = build_kernel()
    nc = _CACHE["nc"]
    in_maps = []
    for i in range(N_CORES):
        m = dict(consts)
        m["x"] = np.ascontiguousarray(x[i*B_LOC:(i+1)*B_LOC])
        in_maps.append(m)
    res = bass_utils.run_bass_kernel_spmd(nc, in_maps, list(range(N_CORES)))
    pooled = np.concatenate([r["pooled"] for r in res.results], axis=0)
    gf = np.concatenate([r["gf"] for r in res.results], axis=0)
    enh = np.concatenate([r["enh"] for r in res.results], axis=0)
    return pooled, gf, enh


# revision 22
# speedup vs baseline: 1.2572x; 1.2572x over previous
"""Trainium2 Bass kernel for DenoisedSignalFeatureExtractor.

Data-parallel over 8 NeuronCores: each core runs 4 of the 32 batch samples
through the full network (conv stem -> 2 mamba blocks -> channel attention ->
pooling). Weights are replicated; all x-dependent compute runs on device.

Layout convention: features on partitions, time on the free dimension.
The selective scan runs as hardware `tensor_tensor_scan` per (d-half, state n)
with exp(A*dt) built on the scalar engine via its per-partition scale operand.
"""
import sys
sys.path.insert(0, "/opt/trn_rl_repo")

import numpy as np
from contextlib import ExitStack

import concourse.bass as bass
import concourse.bacc as bacc
import concourse.mybir as mybir
import concourse.tile as tile
from concourse import bass_utils

F32 = mybir.dt.float32
BF16 = mybir.dt.bfloat16
AF = mybir.ActivationFunctionType
ALU = mybir.AluOpType
AX = mybir.AxisListType

# Model dims (fixed by the problem)
B_TOTAL, L = 32, 1024
D_IN, DM = 32, 128            # input channels, d_model
DI, DS, DC, DTR = 256, 16, 4, 8  # d_inner, d_state, d_conv, dt_rank
POS_LEN = 128
N_CORES = 8
B_LOC = B_TOTAL // N_CORES    # samples per core
TC = 512                      # t-chunk for PSUM-bound matmuls


# ---------------------------------------------------------------- constants

def _np(a):
    return np.asarray(a, dtype=np.float32)


def _interp_pos(pos, Lx):
    P = pos.shape[0]
    src = (np.arange(Lx, dtype=np.float32) + 0.5) * (P / Lx) - 0.5
    src = np.clip(src, 0.0, P - 1.0)
    i0 = np.floor(src).astype(np.int32)
    i1 = np.minimum(i0 + 1, P - 1)
    w = (src - i0)[:, None].astype(np.float32)
    return pos[i0] * (1.0 - w) + pos[i1] * w


def prep_consts(params):
    """All parameter-derived constants, shaped for the kernel's DRAM inputs."""
    c = {}
    pr = params['proj']
    w = _np(pr['w'])                       # (128, 32, 7)
    # stem taps as lhsT [32, 128] per tap, concatenated -> [32, 7*128]
    c['stem_w'] = np.concatenate([w[:, :, k].T for k in range(7)], axis=1).astype(np.float32)
    g, bb = _np(pr['bn_g']), _np(pr['bn_b'])
    c['stem_scale'] = g.reshape(DM, 1)
    c['stem_bias'] = (g * _np(pr['b']) + bb).reshape(DM, 1)
    c['pe'] = _interp_pos(_np(params['pos'][0]), L).T.copy()       # (128, L)

    for l, bp in enumerate(params['blocks']):
        W_in = _np(bp['in_proj'])          # (512, 128)
        c[f'win{l}'] = W_in.T.copy()       # (128, 512) lhsT
        cw = _np(bp['conv_w'])[:, 0, :]                            # (256, 4)
        c[f'convw{l}'] = np.concatenate([cw[:DM], cw[DM:]], axis=1)  # (128, 8)
        cb = _np(bp['conv_b']).reshape(DI, 1)
        c[f'convb{l}'] = np.concatenate([cb[:DM], cb[DM:]], axis=1)  # (128, 2)
        Wx = _np(bp['x_proj'])             # (40, 256)
        wxT = Wx.T                         # (256, 40)
        # pad output rows to partition-aligned bases: dt_lin@0, B@32, C@64
        wxp = np.zeros((DI, 96), np.float32)
        wxp[:, 0:DTR] = wxT[:, 0:DTR]
        wxp[:, 32:32 + DS] = wxT[:, DTR:DTR + DS]
        wxp[:, 64:64 + DS] = wxT[:, DTR + DS:DTR + 2 * DS]
        c[f'wx{l}'] = np.concatenate([wxp[:DM], wxp[DM:]], axis=1)   # (128, 192)
        Wdt = _np(bp['dt_w'])              # (256, 8)
        c[f'wdt{l}'] = Wdt.T.copy()        # (8, 256) lhsT
        db = _np(bp['dt_b']).reshape(DI, 1)
        c[f'dtb{l}'] = np.concatenate([db[:DM], db[DM:]], axis=1)    # (128, 2)
        Am = -np.exp(_np(bp['A_log']))                             # (256, 16)
        c[f'A{l}'] = np.concatenate([Am[:DM], Am[DM:]], axis=1)      # (128, 32)
        dv = _np(bp['D']).reshape(DI, 1)
        c[f'Dv{l}'] = np.concatenate([dv[:DM], dv[DM:]], axis=1)     # (128, 2)
        Wo = _np(bp['out_proj'])           # (128, 256)
        woT = Wo.T                         # (256, 128)
        c[f'wout{l}'] = np.concatenate([woT[:DM], woT[DM:]], axis=1) # (128, 256)
        c[f'ln1g{l}'] = _np(bp['ln1_g']).reshape(DM, 1)
        c[f'ln1b{l}'] = _np(bp['ln1_b']).reshape(DM, 1)
        c[f'ln2g{l}'] = _np(bp['ln2_g']).reshape(DM, 1)
        c[f'ln2b{l}'] = _np(bp['ln2_b']).reshape(DM, 1)
        tw = _np(bp['te_w'])               # (128, 32, 3) groups=4
        WBD = np.zeros((DM, DM, 3), np.float32)
        for gi in range(4):
            WBD[gi*32:(gi+1)*32, gi*32:(gi+1)*32, :] = tw[gi*32:(gi+1)*32]
        c[f'tew{l}'] = np.concatenate([WBD[:, :, k].T for k in range(3)], axis=1)
        tg = _np(bp['te_bn_g'])
        c[f'tes{l}'] = tg.reshape(DM, 1)
        c[f'teb{l}'] = (tg * _np(bp['te_b']) + _np(bp['te_bn_b'])).reshape(DM, 1)

    ca = params['ca']
    c['caw1'] = _np(ca['w1']).T.copy()     # (128, 16) lhsT
    c['caw2'] = _np(ca['w2']).T.copy()     # (16, 128) lhsT
    ew = _np(ca['enh_w'])                  # (128, 4, 3) groups=32
    EBD = np.zeros((DM, DM, 3), np.float32)
    for gi in range(32):
        EBD[gi*4:(gi+1)*4, gi*4:(gi+1)*4, :] = ew[gi*4:(gi+1)*4]
    c['caew'] = np.concatenate([EBD[:, :, k].T for k in range(3)], axis=1)
    cg = _np(ca['bn_g'])
    c['cas'] = cg.reshape(DM, 1)
    c['cab'] = (cg * _np(ca['enh_b']) + _np(ca['bn_b'])).reshape(DM, 1)

    ae = params['ae']
    aw = _np(ae['w'])                      # (128, 128, 3)
    c['aew'] = np.concatenate([aw[:, :, k].T for k in range(3)], axis=1)
    ag = _np(ae['bn_g'])
    c['aes'] = ag.reshape(DM, 1)
    c['aeb'] = (ag * _np(ae['b']) + _np(ae['bn_b'])).reshape(DM, 1)

    # selector matrices: sel_n = [16, 128] with row n all ones, concatenated
    sel = np.zeros((DS, DS * DM), np.float32)
    for n in range(DS):
        sel[n, n*DM:(n+1)*DM] = 1.0
    c['sel'] = sel
    c['ones_col'] = np.ones((DM, 1), np.float32)    # lhsT for partition-sum
    c['ones_row'] = np.ones((1, DM), np.float32)    # lhsT for row-broadcast
    import ml_dtypes
    for k in BF_CONSTS:
        c[k] = c[k].astype(ml_dtypes.bfloat16)
    return c


CONST_SHAPES = None  # filled lazily
BF_CONSTS = {'sel', 'win0', 'win1', 'wx0', 'wx1', 'wdt0', 'wdt1', 'wout0', 'wout1'}


# ---------------------------------------------------------------- kernel body

def build_kernel(b_loc=B_LOC, num_scan_states=DS, nblocks=2, tail=3):
    """Builds the Bacc graph. Returns (nc, const_names)."""
    nc = bacc.Bacc("TRN2", debug=False, enable_asserts=False,
                   detect_race_conditions=False)
    consts = prep_consts_shapes()
    dts = {}
    for name, shape in consts.items():
        dt_ = BF16 if name in BF_CONSTS else F32
        dts[name] = nc.dram_tensor(name, list(shape), dt_, kind="ExternalInput").ap()
    x_in = nc.dram_tensor("x", [b_loc, D_IN, L], F32, kind="ExternalInput").ap()
    o_pooled = nc.dram_tensor("pooled", [b_loc, 3 * DM], F32, kind="ExternalOutput").ap()
    o_gf = nc.dram_tensor("gf", [b_loc, DM], F32, kind="ExternalOutput").ap()
    o_enh = nc.dram_tensor("enh", [b_loc, DM, L], F32, kind="ExternalOutput").ap()

    with tile.TileContext(nc) as tc:
        _body(tc, dts, x_in, o_pooled, o_gf, o_enh, b_loc, num_scan_states, nblocks, tail)
    nc.compile()
    return nc


def prep_consts_shapes():
    global CONST_SHAPES
    if CONST_SHAPES is None:
        CONST_SHAPES = {
            'stem_w': (D_IN, 7 * DM), 'stem_scale': (DM, 1), 'stem_bias': (DM, 1),
            'pe': (DM, L),
            'caw1': (DM, 16), 'caw2': (16, DM), 'caew': (DM, 3 * DM),
            'cas': (DM, 1), 'cab': (DM, 1),
            'aew': (DM, 3 * DM), 'aes': (DM, 1), 'aeb': (DM, 1),
            'sel': (DS, DS * DM), 'ones_col': (DM, 1), 'ones_row': (1, DM),
        }
        for l in range(2):
            CONST_SHAPES.update({
                f'win{l}': (DM, 2 * DI), f'convw{l}': (DM, 2 * DC), f'convb{l}': (DM, 2),
                f'wx{l}': (DM, 2 * 96), f'wdt{l}': (DTR, DI), f'dtb{l}': (DM, 2),
                f'A{l}': (DM, 2 * DS), f'Dv{l}': (DM, 2), f'wout{l}': (DM, 2 * DM),
                f'ln1g{l}': (DM, 1), f'ln1b{l}': (DM, 1),
                f'ln2g{l}': (DM, 1), f'ln2b{l}': (DM, 1),
                f'tew{l}': (DM, 3 * DM), f'tes{l}': (DM, 1), f'teb{l}': (DM, 1),
            })
    return CONST_SHAPES


def _body(tc, dts, x_in, o_pooled, o_gf, o_enh, b_loc, NS, nblocks=2, tail=3):
    nc = tc.nc
    ctx = ExitStack()
    with ctx:
        cpool = ctx.enter_context(tc.tile_pool(name="consts", bufs=1))
        sbp = ctx.enter_context(tc.tile_pool(name="work", bufs=1))
        sb1 = ctx.enter_context(tc.tile_pool(name="big", bufs=1))
        psp = ctx.enter_context(tc.tile_pool(name="ps", bufs=2, space="PSUM"))
        pss = ctx.enter_context(tc.tile_pool(name="ps_small", bufs=2, space="PSUM"))

        # ---- load constants to SBUF once
        ct = {}
        for name, shape in prep_consts_shapes().items():
            dt_ = BF16 if name in BF_CONSTS else F32
            t = cpool.tile(list(shape), dt_, tag=f"c_{name}", name=f"c_{name}")
            nc.sync.dma_start(t[:], dts[name][:])
            ct[name] = t

        NCH = L // TC  # chunks

        eps5 = cpool.tile([1, 1], F32, tag="eps5", name="eps5")
        nc.vector.memset(eps5[:], 1e-5)
        eps8 = cpool.tile([DM, 1], F32, tag="eps8", name="eps8")
        nc.vector.memset(eps8[:], 1e-8)

        def ln(src_tile, g_col, b_col, out_tile):
            """LayerNorm over the partition dim (128) of [128, L]."""
            mrow = sbp.tile([1, L], F32, tag="ln_mrow")
            sq = sbp.tile([DM, L], F32, tag="ln_sq")
            ctr = sbp.tile([DM, L], F32, tag="ln_ctr")
            rrow = sbp.tile([1, L], F32, tag="ln_rrow")
            for c in range(NCH):
                s = slice(c * TC, (c + 1) * TC)
                pm = pss.tile([1, TC], F32, tag="sm_ps")
                nc.tensor.matmul(pm[:], ct['ones_col'][:], src_tile[:, s],
                                 start=True, stop=True)
                nc.scalar.activation(mrow[:, s], pm[:], AF.Copy, scale=1.0 / DM)
                pb = psp.tile([DM, TC], F32, tag="mm_ps")
                nc.tensor.matmul(pb[:], ct['ones_row'][:], mrow[:, s],
                                 start=True, stop=True)
                nc.vector.tensor_sub(ctr[:, s], src_tile[:, s], pb[:])
                nc.scalar.activation(sq[:, s], ctr[:, s], AF.Square)
                pv = pss.tile([1, TC], F32, tag="sm_ps")
                nc.tensor.matmul(pv[:], ct['ones_col'][:], sq[:, s],
                                 start=True, stop=True)
                nc.scalar.activation(rrow[:, s], pv[:], AF.Ln,
                                     scale=1.0 / DM, bias=eps5[:])
                nc.scalar.activation(rrow[:, s], rrow[:, s], AF.Exp, scale=-0.5)
                pr = psp.tile([DM, TC], F32, tag="mm_ps")
                nc.tensor.matmul(pr[:], ct['ones_row'][:], rrow[:, s],
                                 start=True, stop=True)
                nc.vector.tensor_mul(sq[:, s], ctr[:, s], pr[:])
                nc.scalar.activation(out_tile[:, s], sq[:, s], AF.Identity,
                                     scale=g_col[:], bias=b_col[:])

        for b in range(b_loc):
            # ======== stem: conv7 + bn + gelu, + positional add
            xpad = sbp.tile([D_IN, L + 6], F32, tag="xpad")
            nc.vector.memset(xpad[:], 0.0)
            nc.sync.dma_start(xpad[:, 3:3 + L], x_in[b])
            tf = sbp.tile([DM, L], F32, tag="tf")
            for c in range(NCH):
                ps = psp.tile([DM, TC], F32, tag="mm_ps")
                for k in range(7):
                    nc.tensor.matmul(ps[:], ct['stem_w'][:, k*DM:(k+1)*DM],
                                     xpad[:, c*TC + k: c*TC + k + TC],
                                     start=(k == 0), stop=(k == 6))
                nc.scalar.activation(tf[:, c*TC:(c+1)*TC], ps[:], AF.Gelu,
                                     scale=ct['stem_scale'][:], bias=ct['stem_bias'][:])
            nc.vector.tensor_add(tf[:], tf[:], ct['pe'][:])

            # ======== mamba blocks
            for l in range(nblocks):
                tf_bf = sbp.tile([DM, L], BF16, tag="tf_bf")
                nc.scalar.activation(tf_bf[:], tf[:], AF.Copy)
                xr = [sbp.tile([DM, 3 + L], BF16, tag=f"xr{dh}", name=f"xr{dh}") for dh in range(2)]
                zs = [sbp.tile([DM, L], BF16, tag=f"zs{dh}", name=f"zs{dh}") for dh in range(2)]
                for dh in range(2):
                    nc.vector.memset(xr[dh][:, 0:3], 0.0)
                for c in range(NCH):
                    s = slice(c * TC, (c + 1) * TC)
                    for m in range(4):
                        ps = psp.tile([DM, TC], F32, tag="mm_ps")
                        nc.tensor.matmul(ps[:], ct[f'win{l}'][:, m*DM:(m+1)*DM],
                                         tf_bf[:, s], start=True, stop=True)
                        if m < 2:
                            nc.scalar.activation(xr[m][:, 3 + c*TC: 3 + (c+1)*TC],
                                                 ps[:], AF.Copy)
                        else:
                            nc.scalar.activation(zs[m - 2][:, s], ps[:], AF.Silu)

                # causal depthwise conv (k=4) + silu -> u
                u = [sbp.tile([DM, L], BF16, tag=f"u{dh}", name=f"u{dh}") for dh in range(2)]
                dt = [sbp.tile([DM, L], F32, tag=f"dt{dh}", name=f"dt{dh}") for dh in range(2)]
                dtu = [sbp.tile([DM, L], BF16, tag=f"dtu{dh}", name=f"dtu{dh}") for dh in range(2)]
                for dh in range(2):
                    acc0 = sbp.tile([DM, L], BF16, tag="cacc0")
                    acc1 = sbp.tile([DM, L], BF16, tag="cacc1")
                    cw = ct[f'convw{l}']
                    nc.vector.tensor_scalar(acc0[:], xr[dh][:, 0:L],
                                            cw[:, dh*DC:dh*DC+1], None,
                                            op0=ALU.mult)
                    nc.vector.scalar_tensor_tensor(acc1[:], xr[dh][:, 1:1+L],
                                                   cw[:, dh*DC+1:dh*DC+2], acc0[:],
                                                   op0=ALU.mult, op1=ALU.add)
                    nc.vector.scalar_tensor_tensor(acc0[:], xr[dh][:, 2:2+L],
                                                   cw[:, dh*DC+2:dh*DC+3], acc1[:],
                                                   op0=ALU.mult, op1=ALU.add)
                    nc.vector.scalar_tensor_tensor(acc1[:], xr[dh][:, 3:3+L],
                                                   cw[:, dh*DC+3:dh*DC+4], acc0[:],
                                                   op0=ALU.mult, op1=ALU.add)
                    nc.scalar.activation(u[dh][:], acc1[:], AF.Silu,
                                         bias=ct[f'convb{l}'][:, dh:dh+1])

                # x_proj -> dt_lin [8, L], B [16, L], C [16, L]
                dtl = sbp.tile([DTR, L], BF16, tag="dtl")
                Bm = sbp.tile([DS, L], BF16, tag="Bm")
                Cm = sbp.tile([DS, L], BF16, tag="Cm")
                for c in range(NCH):
                    s = slice(c * TC, (c + 1) * TC)
                    ps = pss.tile([96, TC], F32, tag="sm_ps")
                    for dh in range(2):
                        nc.tensor.matmul(ps[:], ct[f'wx{l}'][:, dh*96:(dh+1)*96],
                                         u[dh][:, s], start=(dh == 0), stop=(dh == 1))
                    nc.scalar.activation(dtl[:, s], ps[0:DTR, :], AF.Copy)
                    nc.scalar.activation(Bm[:, s], ps[32:32+DS, :], AF.Copy)
                    nc.scalar.activation(Cm[:, s], ps[64:64+DS, :], AF.Copy)

                # dt = softplus(Wdt @ dt_lin + dt_b) via exp/ln (one table set)
                esbs = []
                for dh in range(2):
                    esb = sbp.tile([DM, L], F32, tag=f"dt_esb{dh}", name=f"esb{dh}")
                    for c in range(NCH):
                        s = slice(c * TC, (c + 1) * TC)
                        ps = psp.tile([DM, TC], F32, tag="mm_ps")
                        nc.tensor.matmul(ps[:], ct[f'wdt{l}'][:, dh*DM:(dh+1)*DM],
                                         dtl[:, s], start=True, stop=True)
                        nc.scalar.activation(esb[:, s], ps[:], AF.Exp,
                                             bias=ct[f'dtb{l}'][:, dh:dh+1])
                    nc.vector.tensor_scalar_add(esb[:], esb[:], 1.0)
                    esbs.append(esb)
                for dh in range(2):
                    nc.scalar.activation(dt[dh][:], esbs[dh][:], AF.Ln)
                    nc.vector.tensor_mul(dtu[dh][:], dt[dh][:], u[dh][:])

                # ---- selective scan: broadcasts shared across dh, bf16 h
                y = [sbp.tile([DM, L], F32, tag=f"y{dh}", name=f"my{dh}") for dh in range(2)]
                hbufs = [sb1.tile([DM, L * NS], BF16, tag=f"hbuf{dh}", name=f"hbuf{dh}")
                         for dh in range(2)]
                hviews = [h.rearrange("p (n t) -> p n t", n=NS) for h in hbufs]
                for n in range(NS):
                    pb = psp.tile([DM, L], F32, tag="bc_ps", bufs=2)
                    for c in range(NCH):
                        s = slice(c * TC, (c + 1) * TC)
                        nc.tensor.matmul(pb[:, s], ct['sel'][:, n*DM:(n+1)*DM],
                                         Bm[:, s], start=True, stop=True)
                    # bf16 SBUF copy of the broadcast so the w-multiply hits
                    # the DVE 2x mode (PSUM operands disqualify fast modes)
                    pbs = sbp.tile([DM, L], BF16, tag="pbs", bufs=2)
                    nc.scalar.activation(pbs[:], pb[:], AF.Copy)
                    for dh in range(2):
                        dA = sbp.tile([DM, L], F32, tag="dA", bufs=3)
                        nc.scalar.activation(
                            dA[:], dt[dh][:], AF.Exp,
                            scale=ct[f'A{l}'][:, dh*DS + n: dh*DS + n + 1])
                        w = sbp.tile([DM, L], BF16, tag="w", bufs=3)
                        nc.vector.tensor_mul(w[:], dtu[dh][:], pbs[:])
                        nc.vector.tensor_tensor_scan(
                            hviews[dh][:, n, :], dA[:], w[:],
                            0.0, op0=ALU.mult, op1=ALU.add)
                for n in range(NS):
                    pc = psp.tile([DM, L], F32, tag="bc_ps", bufs=2)
                    for c in range(NCH):
                        s = slice(c * TC, (c + 1) * TC)
                        nc.tensor.matmul(pc[:, s], ct['sel'][:, n*DM:(n+1)*DM],
                                         Cm[:, s], start=True, stop=True)
                    pcs = sbp.tile([DM, L], BF16, tag="pbs", bufs=2, name="pcs")
                    nc.scalar.activation(pcs[:], pc[:], AF.Copy)
                    for dh in range(2):
                        nc.vector.tensor_mul(hviews[dh][:, n, :], hviews[dh][:, n, :],
                                             pcs[:])
                # pairwise in-place tree sum over n (bf16 2x adds, final add f32)
                for dh in range(2):
                    width = NS
                    while width > 2:
                        width //= 2
                        for i in range(width):
                            nc.vector.tensor_add(hviews[dh][:, i, :],
                                                 hviews[dh][:, 2*i, :],
                                                 hviews[dh][:, 2*i + 1, :])
                    nc.vector.tensor_add(y[dh][:], hviews[dh][:, 0, :],
                                         hviews[dh][:, 1, :])

                # y = (y + u*D) * silu(z); out_proj; ln1; residual; ln2
                mo = sbp.tile([DM, L], F32, tag="mo")
                yb = [sbp.tile([DM, L], BF16, tag=f"yb{dh}", name=f"yb{dh}") for dh in range(2)]
                for dh in range(2):
                    nc.vector.scalar_tensor_tensor(
                        y[dh][:], u[dh][:], ct[f'Dv{l}'][:, dh:dh+1], y[dh][:],
                        op0=ALU.mult, op1=ALU.add)
                    nc.vector.tensor_mul(yb[dh][:], y[dh][:], zs[dh][:])
                for c in range(NCH):
                    s = slice(c * TC, (c + 1) * TC)
                    ps = psp.tile([DM, TC], F32, tag="mm_ps")
                    for dh in range(2):
                        nc.tensor.matmul(ps[:], ct[f'wout{l}'][:, dh*DM:(dh+1)*DM],
                                         yb[dh][:, s], start=(dh == 0), stop=(dh == 1))
                    nc.scalar.activation(mo[:, s], ps[:], AF.Copy)
                y1 = sbp.tile([DM, L], F32, tag="y1")
                ln(mo, ct[f'ln1g{l}'], ct[f'ln1b{l}'], y1)
                nc.vector.tensor_add(y1[:], y1[:], tf[:])
                x2p = sbp.tile([DM, L + 2], F32, tag="x2p")
                nc.vector.memset(x2p[:, 0:1], 0.0)
                nc.vector.memset(x2p[:, L+1:L+2], 0.0)
                x2 = x2p[:, 1:1+L]
                ln(y1, ct[f'ln2g{l}'], ct[f'ln2b{l}'], x2)
                # te conv (block-diag, k=3, pad 1) + gelu + residual
                tfn = sbp.tile([DM, L], F32, tag="ln_sq")
                for c in range(NCH):
                    ps = psp.tile([DM, TC], F32, tag="mm_ps")
                    for k in range(3):
                        nc.tensor.matmul(ps[:], ct[f'tew{l}'][:, k*DM:(k+1)*DM],
                                         x2p[:, c*TC + k: c*TC + k + TC],
                                         start=(k == 0), stop=(k == 2))
                    nc.scalar.activation(tfn[:, c*TC:(c+1)*TC], ps[:], AF.Gelu,
                                         scale=ct[f'tes{l}'][:], bias=ct[f'teb{l}'][:])
                nc.vector.tensor_add(tf[:], tfn[:], x2)

            if tail == 0:
                nc.sync.dma_start(o_enh[b], tf[:])
                zz = sbp.tile([DM, 3], F32, tag="zz")
                nc.vector.memset(zz[:], 0.0)
                nc.sync.dma_start(o_pooled[b, 0:DM], zz[:, 0])
                nc.sync.dma_start(o_pooled[b, DM:2*DM], zz[:, 1])
                nc.sync.dma_start(o_pooled[b, 2*DM:3*DM], zz[:, 2])
                nc.sync.dma_start(o_gf[b], zz[:, 0])
                continue
            # ======== channel attention
            avg = sbp.tile([DM, 1], F32, tag="avg")
            mx = sbp.tile([DM, 1], F32, tag="mx")
            nc.vector.tensor_reduce(avg[:], tf[:], axis=AX.X, op=ALU.add)
            nc.scalar.activation(avg[:], avg[:], AF.Copy, scale=1.0 / L)
            nc.vector.tensor_reduce(mx[:], tf[:], axis=AX.X, op=ALU.max)
            att_ps = pss.tile([DM, 1], F32, tag="sm_ps")
            for i, v in enumerate((avg, mx)):
                ph = pss.tile([16, 1], F32, tag="sm_ps")
                nc.tensor.matmul(ph[:], ct['caw1'][:], v[:], start=True, stop=True)
                hg = sbp.tile([16, 1], F32, tag="cahg")
                nc.scalar.activation(hg[:], ph[:], AF.Gelu)
                nc.tensor.matmul(att_ps[:], ct['caw2'][:], hg[:],
                                 start=(i == 0), stop=(i == 1))
            a_sig = sbp.tile([DM, 1], F32, tag="asig")
            nc.scalar.activation(a_sig[:], att_ps[:], AF.Exp, scale=-1.0)
            nc.vector.tensor_scalar_add(a_sig[:], a_sig[:], 1.0)
            nc.vector.reciprocal(a_sig[:], a_sig[:])
            xap = sbp.tile([DM, L + 2], F32, tag="xr0")
            nc.vector.memset(xap[:, 0:1], 0.0)
            nc.vector.memset(xap[:, L+1:L+2], 0.0)
            nc.vector.tensor_scalar(xap[:, 1:1+L], tf[:], a_sig[:], None, op0=ALU.mult)
            attp = sbp.tile([DM, L + 2], F32, tag="xr1")
            nc.vector.memset(attp[:, 0:1], 0.0)
            nc.vector.memset(attp[:, L+1:L+2], 0.0)
            for c in range(NCH):
                ps = psp.tile([DM, TC], F32, tag="mm_ps")
                for k in range(3):
                    nc.tensor.matmul(ps[:], ct['caew'][:, k*DM:(k+1)*DM],
                                     xap[:, c*TC + k: c*TC + k + TC],
                                     start=(k == 0), stop=(k == 2))
                xe = sbp.tile([DM, TC], F32, tag="dtu0")
                nc.scalar.activation(xe[:], ps[:], AF.Gelu,
                                     scale=ct['cas'][:], bias=ct['cab'][:])
                nc.vector.scalar_tensor_tensor(attp[:, 1 + c*TC: 1 + (c+1)*TC],
                                               xe[:], 0.1,
                                               xap[:, 1 + c*TC: 1 + (c+1)*TC],
                                               op0=ALU.mult, op1=ALU.add)
            # ae conv + gelu + residual -> enh
            enh_t = sbp.tile([DM, L], F32, tag="u0")
            for c in range(NCH):
                ps = psp.tile([DM, TC], F32, tag="mm_ps")
                for k in range(3):
                    nc.tensor.matmul(ps[:], ct['aew'][:, k*DM:(k+1)*DM],
                                     attp[:, c*TC + k: c*TC + k + TC],
                                     start=(k == 0), stop=(k == 2))
                nc.scalar.activation(enh_t[:, c*TC:(c+1)*TC], ps[:], AF.Gelu,
                                     scale=ct['aes'][:], bias=ct['aeb'][:])
            nc.vector.tensor_add(enh_t[:], enh_t[:], attp[:, 1:1+L])
            nc.sync.dma_start(o_enh[b], enh_t[:])
            if tail <= 2:
                zz = sbp.tile([DM, 3], F32, tag="zz")
                nc.vector.memset(zz[:], 0.0)
                nc.sync.dma_start(o_pooled[b, 0:DM], zz[:, 0])
                nc.sync.dma_start(o_pooled[b, DM:2*DM], zz[:, 1])
                nc.sync.dma_start(o_pooled[b, 2*DM:3*DM], zz[:, 2])
                nc.sync.dma_start(o_gf[b], zz[:, 0])
                continue

            # ======== pooling: gf (mean), mx, std
            gf_s = sbp.tile([DM, 1], F32, tag="gfs")
            nc.vector.tensor_reduce(gf_s[:], enh_t[:], axis=AX.X, op=ALU.add)
            gf_m = sbp.tile([DM, 1], F32, tag="gfm")
            nc.scalar.activation(gf_m[:], gf_s[:], AF.Copy, scale=1.0 / L)
            nc.sync.dma_start(o_gf[b], gf_m[:, 0])
            nc.sync.dma_start(o_pooled[b, 0:DM], gf_m[:, 0])
            mx2 = sbp.tile([DM, 1], F32, tag="mx2")
            nc.vector.tensor_reduce(mx2[:], enh_t[:], axis=AX.X, op=ALU.max)
            nc.sync.dma_start(o_pooled[b, DM:2*DM], mx2[:, 0])
            sq_scr = sbp.tile([DM, L], F32, tag="u1")
            nc.scalar.activation(sq_scr[:], enh_t[:], AF.Square)
            ssq = sbp.tile([DM, 1], F32, tag="ssq")
            nc.vector.tensor_reduce(ssq[:], sq_scr[:], axis=AX.X, op=ALU.add)
            gq = sbp.tile([DM, 1], F32, tag="gq")
            nc.scalar.activation(gq[:], gf_m[:], AF.Square)
            var = sbp.tile([DM, 1], F32, tag="var")
            nc.vector.scalar_tensor_tensor(var[:], ssq[:], 1.0 / L, gq[:],
                                           op0=ALU.mult, op1=ALU.subtract)
            nc.vector.tensor_scalar_max(var[:], var[:], 0.0)
            std = sbp.tile([DM, 1], F32, tag="std")
            nc.scalar.activation(std[:], var[:], AF.Ln, bias=eps8[:])
            nc.scalar.activation(std[:], std[:], AF.Exp, scale=0.5)
            nc.sync.dma_start(o_pooled[b, 2*DM:3*DM], std[:, 0])


# ---------------------------------------------------------------- entrypoint

_CACHE = {}


def kernel(x, params):
    x = np.asarray(x, dtype=np.float32)
    consts = prep_consts(params)
    if "nc" not in _CACHE:
        _CACHE["nc"] = build_kernel()
    nc = _CACHE["nc"]
    in_maps = []
    for i in range(N_CORES):
        m = dict(consts)
        m["x"] = np.ascontiguousarray(x[i*B_LOC:(i+1)*B_LOC])
        in_maps.append(m)
    res = bass_utils.run_bass_kernel_spmd(nc, in_maps, list(range(N_CORES)))
    pooled = np.concatenate([r["pooled"] for r in res.results], axis=0)
    gf = np.concatenate([r["gf"] for r in res.results], axis=0)
    enh = np.concatenate([r["enh"] for r in res.results], axis=0)
    return pooled, gf, enh


# revision 23
# speedup vs baseline: 1.2985x; 1.0328x over previous
"""Trainium2 Bass kernel for DenoisedSignalFeatureExtractor.

Data-parallel over 8 NeuronCores: each core runs 4 of the 32 batch samples
through the full network (conv stem -> 2 mamba blocks -> channel attention ->
pooling). Weights are replicated; all x-dependent compute runs on device.

Layout convention: features on partitions, time on the free dimension.
The selective scan runs as hardware `tensor_tensor_scan` per (d-half, state n)
with exp(A*dt) built on the scalar engine via its per-partition scale operand.
"""
import sys
sys.path.insert(0, "/opt/trn_rl_repo")

import numpy as np
from contextlib import ExitStack

import concourse.bass as bass
import concourse.bacc as bacc
import concourse.mybir as mybir
import concourse.tile as tile
from concourse import bass_utils

F32 = mybir.dt.float32
BF16 = mybir.dt.bfloat16
AF = mybir.ActivationFunctionType
ALU = mybir.AluOpType
AX = mybir.AxisListType

# Model dims (fixed by the problem)
B_TOTAL, L = 32, 1024
D_IN, DM = 32, 128            # input channels, d_model
DI, DS, DC, DTR = 256, 16, 4, 8  # d_inner, d_state, d_conv, dt_rank
POS_LEN = 128
N_CORES = 8
B_LOC = B_TOTAL // N_CORES    # samples per core
TC = 512                      # t-chunk for PSUM-bound matmuls


# ---------------------------------------------------------------- constants

def _np(a):
    return np.asarray(a, dtype=np.float32)


def _interp_pos(pos, Lx):
    P = pos.shape[0]
    src = (np.arange(Lx, dtype=np.float32) + 0.5) * (P / Lx) - 0.5
    src = np.clip(src, 0.0, P - 1.0)
    i0 = np.floor(src).astype(np.int32)
    i1 = np.minimum(i0 + 1, P - 1)
    w = (src - i0)[:, None].astype(np.float32)
    return pos[i0] * (1.0 - w) + pos[i1] * w


def prep_consts(params):
    """All parameter-derived constants, shaped for the kernel's DRAM inputs."""
    c = {}
    pr = params['proj']
    w = _np(pr['w'])                       # (128, 32, 7)
    # stem taps as lhsT [32, 128] per tap, concatenated -> [32, 7*128]
    c['stem_w'] = np.concatenate([w[:, :, k].T for k in range(7)], axis=1).astype(np.float32)
    g, bb = _np(pr['bn_g']), _np(pr['bn_b'])
    c['stem_scale'] = g.reshape(DM, 1)
    c['stem_bias'] = (g * _np(pr['b']) + bb).reshape(DM, 1)
    c['pe'] = _interp_pos(_np(params['pos'][0]), L).T.copy()       # (128, L)

    for l, bp in enumerate(params['blocks']):
        W_in = _np(bp['in_proj'])          # (512, 128)
        c[f'win{l}'] = W_in.T.copy()       # (128, 512) lhsT
        cw = _np(bp['conv_w'])[:, 0, :]                            # (256, 4)
        c[f'convw{l}'] = np.concatenate([cw[:DM], cw[DM:]], axis=1)  # (128, 8)
        cb = _np(bp['conv_b']).reshape(DI, 1)
        c[f'convb{l}'] = np.concatenate([cb[:DM], cb[DM:]], axis=1)  # (128, 2)
        Wx = _np(bp['x_proj'])             # (40, 256)
        wxT = Wx.T                         # (256, 40)
        # pad output rows to partition-aligned bases: dt_lin@0, B@32, C@64
        wxp = np.zeros((DI, 96), np.float32)
        wxp[:, 0:DTR] = wxT[:, 0:DTR]
        wxp[:, 32:32 + DS] = wxT[:, DTR:DTR + DS]
        wxp[:, 64:64 + DS] = wxT[:, DTR + DS:DTR + 2 * DS]
        c[f'wx{l}'] = np.concatenate([wxp[:DM], wxp[DM:]], axis=1)   # (128, 192)
        Wdt = _np(bp['dt_w'])              # (256, 8)
        c[f'wdt{l}'] = Wdt.T.copy()        # (8, 256) lhsT
        db = _np(bp['dt_b']).reshape(DI, 1)
        c[f'dtb{l}'] = np.concatenate([db[:DM], db[DM:]], axis=1)    # (128, 2)
        Am = -np.exp(_np(bp['A_log']))                             # (256, 16)
        c[f'A{l}'] = np.concatenate([Am[:DM], Am[DM:]], axis=1)      # (128, 32)
        dv = _np(bp['D']).reshape(DI, 1)
        c[f'Dv{l}'] = np.concatenate([dv[:DM], dv[DM:]], axis=1)     # (128, 2)
        Wo = _np(bp['out_proj'])           # (128, 256)
        woT = Wo.T                         # (256, 128)
        c[f'wout{l}'] = np.concatenate([woT[:DM], woT[DM:]], axis=1) # (128, 256)
        c[f'ln1g{l}'] = _np(bp['ln1_g']).reshape(DM, 1)
        c[f'ln1b{l}'] = _np(bp['ln1_b']).reshape(DM, 1)
        c[f'ln2g{l}'] = _np(bp['ln2_g']).reshape(DM, 1)
        c[f'ln2b{l}'] = _np(bp['ln2_b']).reshape(DM, 1)
        tw = _np(bp['te_w'])               # (128, 32, 3) groups=4
        WBD = np.zeros((DM, DM, 3), np.float32)
        for gi in range(4):
            WBD[gi*32:(gi+1)*32, gi*32:(gi+1)*32, :] = tw[gi*32:(gi+1)*32]
        c[f'tew{l}'] = np.concatenate([WBD[:, :, k].T for k in range(3)], axis=1)
        tg = _np(bp['te_bn_g'])
        c[f'tes{l}'] = tg.reshape(DM, 1)
        c[f'teb{l}'] = (tg * _np(bp['te_b']) + _np(bp['te_bn_b'])).reshape(DM, 1)

    ca = params['ca']
    c['caw1'] = _np(ca['w1']).T.copy()     # (128, 16) lhsT
    c['caw2'] = _np(ca['w2']).T.copy()     # (16, 128) lhsT
    ew = _np(ca['enh_w'])                  # (128, 4, 3) groups=32
    EBD = np.zeros((DM, DM, 3), np.float32)
    for gi in range(32):
        EBD[gi*4:(gi+1)*4, gi*4:(gi+1)*4, :] = ew[gi*4:(gi+1)*4]
    c['caew'] = np.concatenate([EBD[:, :, k].T for k in range(3)], axis=1)
    cg = _np(ca['bn_g'])
    c['cas'] = cg.reshape(DM, 1)
    c['cab'] = (cg * _np(ca['enh_b']) + _np(ca['bn_b'])).reshape(DM, 1)

    ae = params['ae']
    aw = _np(ae['w'])                      # (128, 128, 3)
    c['aew'] = np.concatenate([aw[:, :, k].T for k in range(3)], axis=1)
    ag = _np(ae['bn_g'])
    c['aes'] = ag.reshape(DM, 1)
    c['aeb'] = (ag * _np(ae['b']) + _np(ae['bn_b'])).reshape(DM, 1)

    # selector matrices: sel_n = [16, 128] with row n all ones, concatenated
    sel = np.zeros((DS, DS * DM), np.float32)
    for n in range(DS):
        sel[n, n*DM:(n+1)*DM] = 1.0
    c['sel'] = sel
    c['ones_col'] = np.ones((DM, 1), np.float32)    # lhsT for partition-sum
    c['ones_row'] = np.ones((1, DM), np.float32)    # lhsT for row-broadcast
    import ml_dtypes
    for k in BF_CONSTS:
        c[k] = c[k].astype(ml_dtypes.bfloat16)
    return c


CONST_SHAPES = None  # filled lazily
BF_CONSTS = {'sel', 'win0', 'win1', 'wx0', 'wx1', 'wdt0', 'wdt1', 'wout0', 'wout1'}


# ---------------------------------------------------------------- kernel body

_ACT_TABLES_PATCHED = False


def _patch_act_tables():
    """Route Exp and Ln to the one table set containing both, so the
    softplus/LN exp<->ln chains stop thrashing ACT table loads. Set order
    (= act_func_set_id) is preserved; only membership is filtered."""
    global _ACT_TABLES_PATCHED
    if _ACT_TABLES_PATCHED:
        return
    import concourse.hw_specs as hws
    orig = hws.get_activation_tables

    def patched(module_arch):
        tables = orig(module_arch)
        for name, funcs in tables.items():
            if name != "natural_log_exp_and_others":
                funcs.discard(mybir.ActivationFunctionType.Exp)
                funcs.discard(mybir.ActivationFunctionType.Ln)
        return tables

    hws.get_activation_tables = patched
    bacc.get_activation_tables = patched
    _ACT_TABLES_PATCHED = True


def build_kernel(b_loc=B_LOC, num_scan_states=DS, nblocks=2, tail=3):
    """Builds the Bacc graph. Returns (nc, const_names)."""
    _patch_act_tables()
    nc = bacc.Bacc("TRN2", debug=False, enable_asserts=False,
                   detect_race_conditions=False)
    consts = prep_consts_shapes()
    dts = {}
    for name, shape in consts.items():
        dt_ = BF16 if name in BF_CONSTS else F32
        dts[name] = nc.dram_tensor(name, list(shape), dt_, kind="ExternalInput").ap()
    x_in = nc.dram_tensor("x", [b_loc, D_IN, L], F32, kind="ExternalInput").ap()
    o_pooled = nc.dram_tensor("pooled", [b_loc, 3 * DM], F32, kind="ExternalOutput").ap()
    o_gf = nc.dram_tensor("gf", [b_loc, DM], F32, kind="ExternalOutput").ap()
    o_enh = nc.dram_tensor("enh", [b_loc, DM, L], F32, kind="ExternalOutput").ap()

    with tile.TileContext(nc) as tc:
        _body(tc, dts, x_in, o_pooled, o_gf, o_enh, b_loc, num_scan_states, nblocks, tail)
    nc.compile()
    return nc


def prep_consts_shapes():
    global CONST_SHAPES
    if CONST_SHAPES is None:
        CONST_SHAPES = {
            'stem_w': (D_IN, 7 * DM), 'stem_scale': (DM, 1), 'stem_bias': (DM, 1),
            'pe': (DM, L),
            'caw1': (DM, 16), 'caw2': (16, DM), 'caew': (DM, 3 * DM),
            'cas': (DM, 1), 'cab': (DM, 1),
            'aew': (DM, 3 * DM), 'aes': (DM, 1), 'aeb': (DM, 1),
            'sel': (DS, DS * DM), 'ones_col': (DM, 1), 'ones_row': (1, DM),
        }
        for l in range(2):
            CONST_SHAPES.update({
                f'win{l}': (DM, 2 * DI), f'convw{l}': (DM, 2 * DC), f'convb{l}': (DM, 2),
                f'wx{l}': (DM, 2 * 96), f'wdt{l}': (DTR, DI), f'dtb{l}': (DM, 2),
                f'A{l}': (DM, 2 * DS), f'Dv{l}': (DM, 2), f'wout{l}': (DM, 2 * DM),
                f'ln1g{l}': (DM, 1), f'ln1b{l}': (DM, 1),
                f'ln2g{l}': (DM, 1), f'ln2b{l}': (DM, 1),
                f'tew{l}': (DM, 3 * DM), f'tes{l}': (DM, 1), f'teb{l}': (DM, 1),
            })
    return CONST_SHAPES


def _body(tc, dts, x_in, o_pooled, o_gf, o_enh, b_loc, NS, nblocks=2, tail=3):
    nc = tc.nc
    ctx = ExitStack()
    with ctx:
        cpool = ctx.enter_context(tc.tile_pool(name="consts", bufs=1))
        sbp = ctx.enter_context(tc.tile_pool(name="work", bufs=1))
        sb1 = ctx.enter_context(tc.tile_pool(name="big", bufs=1))
        psp = ctx.enter_context(tc.tile_pool(name="ps", bufs=2, space="PSUM"))
        pss = ctx.enter_context(tc.tile_pool(name="ps_small", bufs=2, space="PSUM"))

        # ---- load constants to SBUF once
        ct = {}
        for name, shape in prep_consts_shapes().items():
            dt_ = BF16 if name in BF_CONSTS else F32
            t = cpool.tile(list(shape), dt_, tag=f"c_{name}", name=f"c_{name}")
            nc.sync.dma_start(t[:], dts[name][:])
            ct[name] = t

        NCH = L // TC  # chunks

        eps5 = cpool.tile([1, 1], F32, tag="eps5", name="eps5")
        nc.vector.memset(eps5[:], 1e-5)
        eps8 = cpool.tile([DM, 1], F32, tag="eps8", name="eps8")
        nc.vector.memset(eps8[:], 1e-8)

        def ln(src_tile, g_col, b_col, out_tile):
            """LayerNorm over the partition dim (128) of [128, L]."""
            mrow = sbp.tile([1, L], F32, tag="ln_mrow")
            sq = sbp.tile([DM, L], F32, tag="ln_sq")
            ctr = sbp.tile([DM, L], F32, tag="ln_ctr")
            rrow = sbp.tile([1, L], F32, tag="ln_rrow")
            for c in range(NCH):
                s = slice(c * TC, (c + 1) * TC)
                pm = pss.tile([1, TC], F32, tag="sm_ps")
                nc.tensor.matmul(pm[:], ct['ones_col'][:], src_tile[:, s],
                                 start=True, stop=True)
                nc.scalar.activation(mrow[:, s], pm[:], AF.Copy, scale=1.0 / DM)
                pb = psp.tile([DM, TC], F32, tag="mm_ps")
                nc.tensor.matmul(pb[:], ct['ones_row'][:], mrow[:, s],
                                 start=True, stop=True)
                nc.vector.tensor_sub(ctr[:, s], src_tile[:, s], pb[:])
                nc.scalar.activation(sq[:, s], ctr[:, s], AF.Square)
                pv = pss.tile([1, TC], F32, tag="sm_ps")
                nc.tensor.matmul(pv[:], ct['ones_col'][:], sq[:, s],
                                 start=True, stop=True)
                nc.scalar.activation(rrow[:, s], pv[:], AF.Ln,
                                     scale=1.0 / DM, bias=eps5[:])
                nc.scalar.activation(rrow[:, s], rrow[:, s], AF.Exp, scale=-0.5)
                pr = psp.tile([DM, TC], F32, tag="mm_ps")
                nc.tensor.matmul(pr[:], ct['ones_row'][:], rrow[:, s],
                                 start=True, stop=True)
                nc.vector.tensor_mul(sq[:, s], ctr[:, s], pr[:])
                nc.scalar.activation(out_tile[:, s], sq[:, s], AF.Identity,
                                     scale=g_col[:], bias=b_col[:])

        for b in range(b_loc):
            # ======== stem: conv7 + bn + gelu, + positional add
            xpad = sbp.tile([D_IN, L + 6], F32, tag="xpad")
            nc.vector.memset(xpad[:], 0.0)
            nc.sync.dma_start(xpad[:, 3:3 + L], x_in[b])
            tf = sbp.tile([DM, L], F32, tag="tf")
            for c in range(NCH):
                ps = psp.tile([DM, TC], F32, tag="mm_ps")
                for k in range(7):
                    nc.tensor.matmul(ps[:], ct['stem_w'][:, k*DM:(k+1)*DM],
                                     xpad[:, c*TC + k: c*TC + k + TC],
                                     start=(k == 0), stop=(k == 6))
                nc.scalar.activation(tf[:, c*TC:(c+1)*TC], ps[:], AF.Gelu,
                                     scale=ct['stem_scale'][:], bias=ct['stem_bias'][:])
            nc.vector.tensor_add(tf[:], tf[:], ct['pe'][:])

            # ======== mamba blocks
            for l in range(nblocks):
                tf_bf = sbp.tile([DM, L], BF16, tag="tf_bf")
                nc.scalar.activation(tf_bf[:], tf[:], AF.Copy)
                xr = [sbp.tile([DM, 3 + L], BF16, tag=f"xr{dh}", name=f"xr{dh}") for dh in range(2)]
                zs = [sbp.tile([DM, L], BF16, tag=f"zs{dh}", name=f"zs{dh}") for dh in range(2)]
                for dh in range(2):
                    nc.vector.memset(xr[dh][:, 0:3], 0.0)
                for c in range(NCH):
                    s = slice(c * TC, (c + 1) * TC)
                    for m in range(4):
                        ps = psp.tile([DM, TC], F32, tag="mm_ps")
                        nc.tensor.matmul(ps[:], ct[f'win{l}'][:, m*DM:(m+1)*DM],
                                         tf_bf[:, s], start=True, stop=True)
                        if m < 2:
                            nc.scalar.activation(xr[m][:, 3 + c*TC: 3 + (c+1)*TC],
                                                 ps[:], AF.Copy)
                        else:
                            nc.scalar.activation(zs[m - 2][:, s], ps[:], AF.Silu)

                # causal depthwise conv (k=4) + silu -> u
                u = [sbp.tile([DM, L], BF16, tag=f"u{dh}", name=f"u{dh}") for dh in range(2)]
                dt = [sbp.tile([DM, L], F32, tag=f"dt{dh}", name=f"dt{dh}") for dh in range(2)]
                dtu = [sbp.tile([DM, L], BF16, tag=f"dtu{dh}", name=f"dtu{dh}") for dh in range(2)]
                for dh in range(2):
                    acc0 = sbp.tile([DM, L], BF16, tag="cacc0")
                    acc1 = sbp.tile([DM, L], BF16, tag="cacc1")
                    cw = ct[f'convw{l}']
                    nc.vector.tensor_scalar(acc0[:], xr[dh][:, 0:L],
                                            cw[:, dh*DC:dh*DC+1], None,
                                            op0=ALU.mult)
                    nc.vector.scalar_tensor_tensor(acc1[:], xr[dh][:, 1:1+L],
                                                   cw[:, dh*DC+1:dh*DC+2], acc0[:],
                                                   op0=ALU.mult, op1=ALU.add)
                    nc.vector.scalar_tensor_tensor(acc0[:], xr[dh][:, 2:2+L],
                                                   cw[:, dh*DC+2:dh*DC+3], acc1[:],
                                                   op0=ALU.mult, op1=ALU.add)
                    nc.vector.scalar_tensor_tensor(acc1[:], xr[dh][:, 3:3+L],
                                                   cw[:, dh*DC+3:dh*DC+4], acc0[:],
                                                   op0=ALU.mult, op1=ALU.add)
                    nc.scalar.activation(u[dh][:], acc1[:], AF.Silu,
                                         bias=ct[f'convb{l}'][:, dh:dh+1])

                # x_proj -> dt_lin [8, L], B [16, L], C [16, L]
                dtl = sbp.tile([DTR, L], BF16, tag="dtl")
                Bm = sbp.tile([DS, L], BF16, tag="Bm")
                Cm = sbp.tile([DS, L], BF16, tag="Cm")
                for c in range(NCH):
                    s = slice(c * TC, (c + 1) * TC)
                    ps = pss.tile([96, TC], F32, tag="sm_ps")
                    for dh in range(2):
                        nc.tensor.matmul(ps[:], ct[f'wx{l}'][:, dh*96:(dh+1)*96],
                                         u[dh][:, s], start=(dh == 0), stop=(dh == 1))
                    nc.scalar.activation(dtl[:, s], ps[0:DTR, :], AF.Copy)
                    nc.scalar.activation(Bm[:, s], ps[32:32+DS, :], AF.Copy)
                    nc.scalar.activation(Cm[:, s], ps[64:64+DS, :], AF.Copy)

                # dt = softplus(Wdt @ dt_lin + dt_b) via exp/ln (one table set)
                esbs = []
                for dh in range(2):
                    esb = sbp.tile([DM, L], F32, tag=f"dt_esb{dh}", name=f"esb{dh}")
                    for c in range(NCH):
                        s = slice(c * TC, (c + 1) * TC)
                        ps = psp.tile([DM, TC], F32, tag="mm_ps")
                        nc.tensor.matmul(ps[:], ct[f'wdt{l}'][:, dh*DM:(dh+1)*DM],
                                         dtl[:, s], start=True, stop=True)
                        nc.scalar.activation(esb[:, s], ps[:], AF.Exp,
                                             bias=ct[f'dtb{l}'][:, dh:dh+1])
                    nc.vector.tensor_scalar_add(esb[:], esb[:], 1.0)
                    esbs.append(esb)
                for dh in range(2):
                    nc.scalar.activation(dt[dh][:], esbs[dh][:], AF.Ln)
                    nc.vector.tensor_mul(dtu[dh][:], dt[dh][:], u[dh][:])

                # ---- selective scan: broadcasts shared across dh, bf16 h
                y = [sbp.tile([DM, L], F32, tag=f"y{dh}", name=f"my{dh}") for dh in range(2)]
                hbufs = [sb1.tile([DM, L * NS], BF16, tag=f"hbuf{dh}", name=f"hbuf{dh}")
                         for dh in range(2)]
                hviews = [h.rearrange("p (n t) -> p n t", n=NS) for h in hbufs]
                for n in range(NS):
                    pb = psp.tile([DM, L], F32, tag="bc_ps", bufs=2)
                    for c in range(NCH):
                        s = slice(c * TC, (c + 1) * TC)
                        nc.tensor.matmul(pb[:, s], ct['sel'][:, n*DM:(n+1)*DM],
                                         Bm[:, s], start=True, stop=True)
                    # bf16 SBUF copy of the broadcast so the w-multiply hits
                    # the DVE 2x mode (PSUM operands disqualify fast modes)
                    pbs = sbp.tile([DM, L], BF16, tag="pbs", bufs=2)
                    nc.scalar.activation(pbs[:], pb[:], AF.Copy)
                    for dh in range(2):
                        dA = sbp.tile([DM, L], F32, tag="dA", bufs=3)
                        nc.scalar.activation(
                            dA[:], dt[dh][:], AF.Exp,
                            scale=ct[f'A{l}'][:, dh*DS + n: dh*DS + n + 1])
                        w = sbp.tile([DM, L], BF16, tag="w", bufs=3)
                        nc.vector.tensor_mul(w[:], dtu[dh][:], pbs[:])
                        nc.vector.tensor_tensor_scan(
                            hviews[dh][:, n, :], dA[:], w[:],
                            0.0, op0=ALU.mult, op1=ALU.add)
                for n in range(NS):
                    pc = psp.tile([DM, L], F32, tag="bc_ps", bufs=2)
                    for c in range(NCH):
                        s = slice(c * TC, (c + 1) * TC)
                        nc.tensor.matmul(pc[:, s], ct['sel'][:, n*DM:(n+1)*DM],
                                         Cm[:, s], start=True, stop=True)
                    pcs = sbp.tile([DM, L], BF16, tag="pbs", bufs=2, name="pcs")
                    nc.scalar.activation(pcs[:], pc[:], AF.Copy)
                    for dh in range(2):
                        nc.vector.tensor_mul(hviews[dh][:, n, :], hviews[dh][:, n, :],
                                             pcs[:])
                # pairwise in-place tree sum over n (bf16 2x adds, final add f32)
                for dh in range(2):
                    width = NS
                    while width > 2:
                        width //= 2
                        for i in range(width):
                            nc.vector.tensor_add(hviews[dh][:, i, :],
                                                 hviews[dh][:, 2*i, :],
                                                 hviews[dh][:, 2*i + 1, :])
                    nc.vector.tensor_add(y[dh][:], hviews[dh][:, 0, :],
                                         hviews[dh][:, 1, :])

                # y = (y + u*D) * silu(z); out_proj; ln1; residual; ln2
                mo = sbp.tile([DM, L], F32, tag="mo")
                yb = [sbp.tile([DM, L], BF16, tag=f"yb{dh}", name=f"yb{dh}") for dh in range(2)]
                for dh in range(2):
                    nc.vector.scalar_tensor_tensor(
                        y[dh][:], u[dh][:], ct[f'Dv{l}'][:, dh:dh+1], y[dh][:],
                        op0=ALU.mult, op1=ALU.add)
                    nc.vector.tensor_mul(yb[dh][:], y[dh][:], zs[dh][:])
                for c in range(NCH):
                    s = slice(c * TC, (c + 1) * TC)
                    ps = psp.tile([DM, TC], F32, tag="mm_ps")
                    for dh in range(2):
                        nc.tensor.matmul(ps[:], ct[f'wout{l}'][:, dh*DM:(dh+1)*DM],
                                         yb[dh][:, s], start=(dh == 0), stop=(dh == 1))
                    nc.scalar.activation(mo[:, s], ps[:], AF.Copy)
                y1 = sbp.tile([DM, L], F32, tag="y1")
                ln(mo, ct[f'ln1g{l}'], ct[f'ln1b{l}'], y1)
                nc.vector.tensor_add(y1[:], y1[:], tf[:])
                x2p = sbp.tile([DM, L + 2], F32, tag="x2p")
                nc.vector.memset(x2p[:, 0:1], 0.0)
                nc.vector.memset(x2p[:, L+1:L+2], 0.0)
                x2 = x2p[:, 1:1+L]
                ln(y1, ct[f'ln2g{l}'], ct[f'ln2b{l}'], x2)
                # te conv (block-diag, k=3, pad 1) + gelu + residual
                tfn = sbp.tile([DM, L], F32, tag="ln_sq")
                for c in range(NCH):
                    ps = psp.tile([DM, TC], F32, tag="mm_ps")
                    for k in range(3):
                        nc.tensor.matmul(ps[:], ct[f'tew{l}'][:, k*DM:(k+1)*DM],
                                         x2p[:, c*TC + k: c*TC + k + TC],
                                         start=(k == 0), stop=(k == 2))
                    nc.scalar.activation(tfn[:, c*TC:(c+1)*TC], ps[:], AF.Gelu,
                                         scale=ct[f'tes{l}'][:], bias=ct[f'teb{l}'][:])
                nc.vector.tensor_add(tf[:], tfn[:], x2)

            if tail == 0:
                nc.sync.dma_start(o_enh[b], tf[:])
                zz = sbp.tile([DM, 3], F32, tag="zz")
                nc.vector.memset(zz[:], 0.0)
                nc.sync.dma_start(o_pooled[b, 0:DM], zz[:, 0])
                nc.sync.dma_start(o_pooled[b, DM:2*DM], zz[:, 1])
                nc.sync.dma_start(o_pooled[b, 2*DM:3*DM], zz[:, 2])
                nc.sync.dma_start(o_gf[b], zz[:, 0])
                continue
            # ======== channel attention
            avg = sbp.tile([DM, 1], F32, tag="avg")
            mx = sbp.tile([DM, 1], F32, tag="mx")
            nc.vector.tensor_reduce(avg[:], tf[:], axis=AX.X, op=ALU.add)
            nc.scalar.activation(avg[:], avg[:], AF.Copy, scale=1.0 / L)
            nc.vector.tensor_reduce(mx[:], tf[:], axis=AX.X, op=ALU.max)
            att_ps = pss.tile([DM, 1], F32, tag="sm_ps")
            for i, v in enumerate((avg, mx)):
                ph = pss.tile([16, 1], F32, tag="sm_ps")
                nc.tensor.matmul(ph[:], ct['caw1'][:], v[:], start=True, stop=True)
                hg = sbp.tile([16, 1], F32, tag="cahg")
                nc.scalar.activation(hg[:], ph[:], AF.Gelu)
                nc.tensor.matmul(att_ps[:], ct['caw2'][:], hg[:],
                                 start=(i == 0), stop=(i == 1))
            a_sig = sbp.tile([DM, 1], F32, tag="asig")
            nc.scalar.activation(a_sig[:], att_ps[:], AF.Exp, scale=-1.0)
            nc.vector.tensor_scalar_add(a_sig[:], a_sig[:], 1.0)
            nc.vector.reciprocal(a_sig[:], a_sig[:])
            xap = sbp.tile([DM, L + 2], F32, tag="xr0")
            nc.vector.memset(xap[:, 0:1], 0.0)
            nc.vector.memset(xap[:, L+1:L+2], 0.0)
            nc.vector.tensor_scalar(xap[:, 1:1+L], tf[:], a_sig[:], None, op0=ALU.mult)
            attp = sbp.tile([DM, L + 2], F32, tag="xr1")
            nc.vector.memset(attp[:, 0:1], 0.0)
            nc.vector.memset(attp[:, L+1:L+2], 0.0)
            for c in range(NCH):
                ps = psp.tile([DM, TC], F32, tag="mm_ps")
                for k in range(3):
                    nc.tensor.matmul(ps[:], ct['caew'][:, k*DM:(k+1)*DM],
                                     xap[:, c*TC + k: c*TC + k + TC],
                                     start=(k == 0), stop=(k == 2))
                xe = sbp.tile([DM, TC], F32, tag="dtu0")
                nc.scalar.activation(xe[:], ps[:], AF.Gelu,
                                     scale=ct['cas'][:], bias=ct['cab'][:])
                nc.vector.scalar_tensor_tensor(attp[:, 1 + c*TC: 1 + (c+1)*TC],
                                               xe[:], 0.1,
                                               xap[:, 1 + c*TC: 1 + (c+1)*TC],
                                               op0=ALU.mult, op1=ALU.add)
            # ae conv + gelu + residual -> enh
            enh_t = sbp.tile([DM, L], F32, tag="u0")
            for c in range(NCH):
                ps = psp.tile([DM, TC], F32, tag="mm_ps")
                for k in range(3):
                    nc.tensor.matmul(ps[:], ct['aew'][:, k*DM:(k+1)*DM],
                                     attp[:, c*TC + k: c*TC + k + TC],
                                     start=(k == 0), stop=(k == 2))
                nc.scalar.activation(enh_t[:, c*TC:(c+1)*TC], ps[:], AF.Gelu,
                                     scale=ct['aes'][:], bias=ct['aeb'][:])
            nc.vector.tensor_add(enh_t[:], enh_t[:], attp[:, 1:1+L])
            nc.sync.dma_start(o_enh[b], enh_t[:])
            if tail <= 2:
                zz = sbp.tile([DM, 3], F32, tag="zz")
                nc.vector.memset(zz[:], 0.0)
                nc.sync.dma_start(o_pooled[b, 0:DM], zz[:, 0])
                nc.sync.dma_start(o_pooled[b, DM:2*DM], zz[:, 1])
                nc.sync.dma_start(o_pooled[b, 2*DM:3*DM], zz[:, 2])
                nc.sync.dma_start(o_gf[b], zz[:, 0])
                continue

            # ======== pooling: gf (mean), mx, std
            gf_s = sbp.tile([DM, 1], F32, tag="gfs")
            nc.vector.tensor_reduce(gf_s[:], enh_t[:], axis=AX.X, op=ALU.add)
            gf_m = sbp.tile([DM, 1], F32, tag="gfm")
            nc.scalar.activation(gf_m[:], gf_s[:], AF.Copy, scale=1.0 / L)
            nc.sync.dma_start(o_gf[b], gf_m[:, 0])
            nc.sync.dma_start(o_pooled[b, 0:DM], gf_m[:, 0])
            mx2 = sbp.tile([DM, 1], F32, tag="mx2")
            nc.vector.tensor_reduce(mx2[:], enh_t[:], axis=AX.X, op=ALU.max)
            nc.sync.dma_start(o_pooled[b, DM:2*DM], mx2[:, 0])
            sq_scr = sbp.tile([DM, L], F32, tag="u1")
            nc.scalar.activation(sq_scr[:], enh_t[:], AF.Square)
            ssq = sbp.tile([DM, 1], F32, tag="ssq")
            nc.vector.tensor_reduce(ssq[:], sq_scr[:], axis=AX.X, op=ALU.add)
            gq = sbp.tile([DM, 1], F32, tag="gq")
            nc.scalar.activation(gq[:], gf_m[:], AF.Square)
            var = sbp.tile([DM, 1], F32, tag="var")
            nc.vector.scalar_tensor_tensor(var[:], ssq[:], 1.0 / L, gq[:],
                                           op0=ALU.mult, op1=ALU.subtract)
            nc.vector.tensor_scalar_max(var[:], var[:], 0.0)
            std = sbp.tile([DM, 1], F32, tag="std")
            nc.scalar.activation(std[:], var[:], AF.Ln, bias=eps8[:])
            nc.scalar.activation(std[:], std[:], AF.Exp, scale=0.5)
            nc.sync.dma_start(o_pooled[b, 2*DM:3*DM], std[:, 0])


# ---------------------------------------------------------------- entrypoint

_CACHE = {}


def kernel(x, params):
    x = np.asarray(x, dtype=np.float32)
    consts = prep_consts(params)
    if "nc" not in _CACHE:
        _CACHE["nc"] = build_kernel()
    nc = _CACHE["nc"]
    in_maps = []
    for i in range(N_CORES):
        m = dict(consts)
        m["x"] = np.ascontiguousarray(x[i*B_LOC:(i+1)*B_LOC])
        in_maps.append(m)
    res = bass_utils.run_bass_kernel_spmd(nc, in_maps, list(range(N_CORES)))
    pooled = np.concatenate([r["pooled"] for r in res.results], axis=0)
    gf = np.concatenate([r["gf"] for r in res.results], axis=0)
    enh = np.concatenate([r["enh"] for r in res.results], axis=0)
    return pooled, gf, enh
